# revision 4
# baseline (speedup 1.0000x reference)
"""ContrastiveLoss (ragged cross-attention i2t scores + hinge) on 8 trn2 cores.

Strategy (see spec sharding_hint): shard captions across cores. Each core
holds all image regions and a 1/8 slice of captions (dealt from a global
sort by caption length so all cores execute an identical program), computes
its [I, C/8] score columns on-device, and the tiny O(B^2) hinge reduction
is done on host after gathering the column slices.

Device kernel per core:
  - single packed input DMA: [128, TOT] fp32 holding
      * im as lhsT in two partition halves ([64, I*O/2] each)
      * packed caption words (valid words only, padded to canonical rank
        lengths with a duplicate of the last valid word)
      * per-row-tile "ones" matrices with 1/(im_l+eps) folded in
  - for each of I*O/128 row tiles:
      matmul [64,128]^T @ [64, caps*len] -> PSUM (<=512-col banks)
      segmented reduce_max over word groups (4D AP) -> maxed [128, C/8]
      ones-matmul accumulates region-sums into per-phase score PSUM
  - score tiles copied to SBUF (ScalarE) and DMA'd out.
"""

import numpy as np

EPS = 1e-6
MARGIN = 0.2
NCORES = 8
P = 128          # partitions / row-tile height
BANK = 512       # fp32 cols per PSUM bank
TBANKS = 3       # banks per mm psum tile

_CACHE = {}


def _plan_tiles(CL):
    """Greedy caption tiles from canonical per-rank lengths (desc sorted).
    Returns list of (j0, Wt, k, kbs, n): ranks [j0, j0+n), padded len Wt,
    k caps per full bank, kbs caps per bank."""
    ranks = len(CL)
    tiles = []
    j = 0
    while j < ranks:
        Wt = int(CL[j])
        k = BANK // Wt
        n = min(TBANKS * k, ranks - j)
        nb = (n + k - 1) // k
        kbs = [min(k, n - b * k) for b in range(nb)]
        tiles.append((j, Wt, k, kbs, n))
        j += n
    return tiles


def _build_program(I, O, D, Cc, tiles, SC):
    """Build the SPMD Bass program (identical for all cores).
    Cc = captions per core."""
    import concourse.bacc as bacc
    import concourse.mybir as mybir
    from concourse import tile as tl

    f32 = mybir.dt.float32
    ROWS = I * O
    NRT = ROWS // P                      # row tiles
    HALF = ROWS // 2                     # im cols per partition-half
    PH = (np.lcm(O, P) // P)             # row tiles per image phase (9)
    NIPH = (PH * P) // O                 # images per phase (32)
    IMB, SB_, OB = 0, HALF, HALF + SC    # col bases: im, s, ones
    TOT = HALF + SC + NRT * NIPH

    nc = bacc.Bacc()
    inp = nc.dram_tensor("inp", [P, TOT], f32, kind="ExternalInput")
    scores = nc.dram_tensor("scores", [I, Cc], f32, kind="ExternalOutput")

    with tl.TileContext(nc) as tc:
        with (
            tc.tile_pool(name="sb", bufs=1) as sb,
            tc.tile_pool(name="mx", bufs=3) as mx,
            tc.tile_pool(name="sco", bufs=2) as sco,
            tc.tile_pool(name="ps", bufs=2, space="PSUM") as ps,
            tc.tile_pool(name="ps2", bufs=2, space="PSUM") as ps2,
        ):
            big = sb.tile([P, TOT], f32)
            nc.sync.dma_start(out=big[:], in_=inp[:])

            spsum = None
            pending_ones = None  # (rt, maxed) emitted after next rt's first MMs

            for rt in range(NRT):
                if rt < NRT // 2:
                    pbase = 0
                    imw = big[0:64, IMB + rt * P : IMB + (rt + 1) * P]
                else:
                    pbase = 64
                    c0 = IMB + (rt - NRT // 2) * P
                    imw = big[64:128, c0 : c0 + P]
                maxed = mx.tile([P, Cc], f32)
                scol = SB_
                for ti, (j0, Wt, k, kbs, n) in enumerate(tiles):
                    psum = ps.tile([P, TBANKS * BANK], f32)
                    for b, kb in enumerate(kbs):
                        nc.tensor.matmul(
                            psum[:, b * BANK : b * BANK + kb * Wt],
                            lhsT=imw,
                            rhs=big[pbase : pbase + 64, scol : scol + kb * Wt],
                            start=True,
                            stop=True,
                        )
                        scol += kb * Wt
                    if ti == 0 and pending_ones is not None:
                        _emit_ones(nc, big, OB, pending_ones, spsum, PH, NIPH, Cc)
                        pending_ones = None
                    nfull = n // k
                    if nfull:
                        src = (
                            psum[:, : nfull * BANK]
                            .rearrange("p (b f) -> p b f", f=BANK)[:, :, : k * Wt]
                            .rearrange("p b (c w) -> p b c w", w=Wt)
                        )
                        outap = maxed[:, j0 : j0 + nfull * k].rearrange(
                            "p (b c) -> p b c", c=k
                        )
                        nc.vector.reduce_max(outap, src, axis=mybir.AxisListType.X)
                    if n % k:
                        kb = n - nfull * k
                        src = psum[
                            :, nfull * BANK : nfull * BANK + kb * Wt
                        ].rearrange("p (c w) -> p c w", w=Wt)
                        nc.vector.reduce_max(
                            maxed[:, j0 + nfull * k : j0 + n],
                            src,
                            axis=mybir.AxisListType.X,
                        )
                if rt % PH == 0:
                    spsum = ps2.tile([NIPH, Cc], f32)
                pending_ones = (rt, maxed)
                if rt % PH == PH - 1:
                    _emit_ones(nc, big, OB, pending_ones, spsum, PH, NIPH, Cc)
                    pending_ones = None
                    ssb = sco.tile([NIPH, Cc], f32)
                    nc.scalar.copy(ssb[:], spsum[:])
                    p0 = (rt // PH) * NIPH
                    nc.sync.dma_start(
                        out=scores[p0 : p0 + NIPH, :], in_=ssb[:]
                    )
    nc.finalize()
    return nc


def _emit_ones(nc, big, OB, pending, spsum, PH, NIPH, Cc):
    rt, maxed = pending
    nc.tensor.matmul(
        spsum[:],
        lhsT=big[:, OB + rt * NIPH : OB + (rt + 1) * NIPH],
        rhs=maxed[:],
        start=(rt % PH == 0),
        stop=(rt % PH == PH - 1),
    )


def _prepare(im, s, im_l, s_l):
    I, O, D = im.shape
    C, W, _ = s.shape
    Cc = C // NCORES
    ROWS = I * O

    order = np.argsort(-s_l, kind="stable")
    CL = s_l[order[0::NCORES]].astype(np.int64)  # canonical rank lengths
    tiles = _plan_tiles(CL)
    SC = sum(kb * Wt for (_, Wt, _, kbs, _) in tiles for kb in kbs)

    HALF = ROWS // 2
    PH = int(np.lcm(O, P) // P)
    NIPH = (PH * P) // O
    NRT = ROWS // P
    OB = HALF + SC
    TOT = HALF + SC + NRT * NIPH

    base = np.zeros((P, TOT), dtype=np.float32)
    im_flat = np.ascontiguousarray(im.reshape(ROWS, D))
    base[0:64, :HALF] = im_flat[:HALF].T
    base[64:128, :HALF] = im_flat[HALF:].T

    # ones region: [128, NRT*NIPH]; col rt*NIPH + j = image j of phase rt//PH
    inv_l = (1.0 / (im_l.astype(np.float64) + EPS)).astype(np.float32)
    g = np.arange(ROWS)
    img = g // O
    rtv = g // P
    rv = g % P
    jv = img - NIPH * (rtv // PH)
    ones = np.zeros((NRT, P, NIPH), dtype=np.float32)
    ones[rtv, rv, jv] = inv_l[img]
    base[:, OB:] = ones.transpose(1, 0, 2).reshape(P, NRT * NIPH)

    in_maps = []
    for m in range(NCORES):
        bm = base.copy()
        scol = HALF
        for (j0, Wt, k, kbs, n) in tiles:
            ids = order[NCORES * np.arange(j0, j0 + n) + m]
            lens = s_l[ids]
            widx = np.minimum(np.arange(Wt)[None, :], (lens - 1)[:, None])
            blk = s[ids[:, None], widx, :]          # [n, Wt, D]
            blkT = blk.reshape(n * Wt, 64).T
            bm[0:64, scol : scol + n * Wt] = blkT      # for row-tile half 0
            bm[64:128, scol : scol + n * Wt] = blkT    # for row-tile half 1
            scol += n * Wt
        in_maps.append({"inp": bm})
    return order, tiles, SC, in_maps


def _loss_from_scores(full):
    d = np.diag(full).copy()
    t1 = np.maximum(MARGIN + full - d[:, None], 0.0)
    t2 = np.maximum(MARGIN + full - d[None, :], 0.0)
    np.fill_diagonal(t1, 0.0)
    np.fill_diagonal(t2, 0.0)
    return np.float32(t1.mean(dtype=np.float64) + t2.mean(dtype=np.float64))


def kernel(im, s, im_l, s_l):
    from concourse.bass_utils import run_bass_kernel_spmd

    im = np.asarray(im, dtype=np.float32)
    s = np.asarray(s, dtype=np.float32)
    im_l = np.asarray(im_l, dtype=np.int32)
    s_l = np.asarray(s_l, dtype=np.int32)
    I, O, D = im.shape
    C, W, _ = s.shape
    Cc = C // NCORES

    order, tiles, SC, in_maps = _prepare(im, s, im_l, s_l)

    key = (I, O, D, C, W, tuple(int(t[1]) for t in tiles), tuple(int(t[4]) for t in tiles))
    if key not in _CACHE:
        _CACHE[key] = _build_program(I, O, D, Cc, tiles, SC)
    nc = _CACHE[key]

    res = run_bass_kernel_spmd(nc, in_maps, list(range(NCORES)))

    full = np.empty((I, C), dtype=np.float32)
    jr = np.arange(Cc)
    for m in range(NCORES):
        full[:, order[NCORES * jr + m]] = res.results[m]["scores"]
    return _loss_from_scores(full)


# revision 12
# speedup vs baseline: 1633.8757x; 1633.8757x over previous
"""ContrastiveLoss (ragged cross-attention i2t scores + hinge) on 8 trn2 cores.

Strategy (see spec sharding_hint): shard captions across cores. Each core
holds all image regions and a 1/8 slice of captions (dealt from a global
sort by caption length so all cores execute an identical program), computes
its [I, C/8] score columns on-device, and the tiny O(B^2) hinge reduction
is done on host after gathering the column slices.

Device kernel per core:
  - single packed input DMA: [128, TOT] fp32 holding
      * im as lhsT in two partition halves ([64, I*O/2] each)
      * packed caption words (valid words only, padded to canonical rank
        lengths with a duplicate of the last valid word)
      * per-row-tile "ones" matrices with 1/(im_l+eps) folded in
  - for each of I*O/128 row tiles:
      matmul [64,128]^T @ [64, caps*len] -> PSUM (<=512-col banks)
      segmented reduce_max over word groups (4D AP) -> maxed [128, C/8]
      ones-matmul accumulates region-sums into per-phase score PSUM
  - score tiles copied to SBUF (ScalarE) and DMA'd out.
"""

import numpy as np

EPS = 1e-6
MARGIN = 0.2
NCORES = 8
P = 128          # partitions / row-tile height
BANK = 512       # fp32 cols per PSUM bank
TBANKS = 3       # banks per mm psum tile

_CACHE = {}


def _plan_tiles(CL):
    """Greedy caption tiles from canonical per-rank lengths (desc sorted).
    Returns list of (j0, Wt, k, kbs, n): ranks [j0, j0+n), padded len Wt,
    k caps per full bank, kbs caps per bank."""
    ranks = len(CL)
    tiles = []
    j = 0
    while j < ranks:
        Wt = int(CL[j])
        k = BANK // Wt
        n = min(TBANKS * k, ranks - j)
        nb = (n + k - 1) // k
        kbs = [min(k, n - b * k) for b in range(nb)]
        tiles.append((j, Wt, k, kbs, n))
        j += n
    return tiles


def _build_program(I, O, D, Cc, tiles, SC):
    """Build the SPMD Bass program (identical for all cores).
    Cc = captions per core."""
    import concourse.bacc as bacc
    import concourse.mybir as mybir
    from concourse import tile as tl

    f32 = mybir.dt.float32
    bf16 = mybir.dt.bfloat16
    ROWS = I * O
    NRT = ROWS // P                      # row tiles
    HALFB = ROWS // 2                    # im bf16 cols per partition-half
    HALFF = HALFB // 2                   # ... in fp32 col units
    SCE = SC + (SC % 2)                  # s region bf16 cols (even)
    SCF = SCE // 2
    PH = (np.lcm(O, P) // P)             # row tiles per image phase (9)
    NIPH = (PH * P) // O                 # images per phase (32)
    OB = HALFF + SCF                     # ones base (fp32 cols)
    TOT = HALFF + SCF + NRT * NIPH
    SBB = 2 * HALFF                      # s base in bf16 cols

    nc = bacc.Bacc()
    inp = nc.dram_tensor("inp", [P, TOT], f32, kind="ExternalInput")
    scores = nc.dram_tensor("scores", [I, Cc], f32, kind="ExternalOutput")

    with tl.TileContext(nc) as tc:
        with (
            tc.tile_pool(name="sb", bufs=1) as sb,
            tc.tile_pool(name="mx", bufs=3) as mx,
            tc.tile_pool(name="sco", bufs=2) as sco,
            tc.tile_pool(name="ps", bufs=2, space="PSUM") as ps,
            tc.tile_pool(name="ps2", bufs=2, space="PSUM") as ps2,
        ):
            big = sb.tile([P, TOT], f32)
            nc.sync.dma_start(out=big[:], in_=inp[:])
            bigb = big[:].bitcast(bf16)          # [P, 2*TOT] bf16 view

            spsum = None
            pending_ones = None  # (rt, maxed) emitted after next rt's first MMs

            for rt in range(NRT):
                if rt < NRT // 2:
                    pbase = 0
                    imw = bigb[0:64, rt * P : (rt + 1) * P]
                else:
                    pbase = 64
                    c0 = (rt - NRT // 2) * P
                    imw = bigb[64:128, c0 : c0 + P]
                maxed = mx.tile([P, Cc], f32)
                scol = SBB
                for ti, (j0, Wt, k, kbs, n) in enumerate(tiles):
                    psum = ps.tile([P, TBANKS * BANK], f32)
                    for b, kb in enumerate(kbs):
                        nc.tensor.matmul(
                            psum[:, b * BANK : b * BANK + kb * Wt],
                            lhsT=imw,
                            rhs=bigb[pbase : pbase + 64, scol : scol + kb * Wt],
                            start=True,
                            stop=True,
                        )
                        scol += kb * Wt
                    if ti == 0 and pending_ones is not None:
                        _emit_ones(nc, big, OB, pending_ones, spsum, PH, NIPH, Cc)
                        pending_ones = None
                    nfull = n // k
                    if nfull:
                        src = (
                            psum[:, : nfull * BANK]
                            .rearrange("p (b f) -> p b f", f=BANK)[:, :, : k * Wt]
                            .rearrange("p b (c w) -> p b c w", w=Wt)
                        )
                        outap = maxed[:, j0 : j0 + nfull * k].rearrange(
                            "p (b c) -> p b c", c=k
                        )
                        nc.vector.reduce_max(outap, src, axis=mybir.AxisListType.X)
                    if n % k:
                        kb = n - nfull * k
                        src = psum[
                            :, nfull * BANK : nfull * BANK + kb * Wt
                        ].rearrange("p (c w) -> p c w", w=Wt)
                        nc.vector.reduce_max(
                            maxed[:, j0 + nfull * k : j0 + n],
                            src,
                            axis=mybir.AxisListType.X,
                        )
                if rt % PH == 0:
                    spsum = ps2.tile([NIPH, Cc], f32)
                pending_ones = (rt, maxed)
                if rt % PH == PH - 1:
                    _emit_ones(nc, big, OB, pending_ones, spsum, PH, NIPH, Cc)
                    pending_ones = None
                    ssb = sco.tile([NIPH, Cc], f32)
                    nc.scalar.copy(ssb[:], spsum[:])
                    p0 = (rt // PH) * NIPH
                    nc.sync.dma_start(
                        out=scores[p0 : p0 + NIPH, :], in_=ssb[:]
                    )
    nc.finalize()
    return nc


def _emit_ones(nc, big, OB, pending, spsum, PH, NIPH, Cc):
    rt, maxed = pending
    nc.tensor.matmul(
        spsum[:],
        lhsT=big[:, OB + rt * NIPH : OB + (rt + 1) * NIPH],
        rhs=maxed[:],
        start=(rt % PH == 0),
        stop=(rt % PH == PH - 1),
    )


def _prepare(im, s, im_l, s_l):
    I, O, D = im.shape
    C, W, _ = s.shape
    Cc = C // NCORES
    ROWS = I * O

    order = np.argsort(-s_l, kind="stable")
    CL = s_l[order[0::NCORES]].astype(np.int64)  # canonical rank lengths
    tiles = _plan_tiles(CL)
    SC = sum(kb * Wt for (_, Wt, _, kbs, _) in tiles for kb in kbs)

    import ml_dtypes

    bf = ml_dtypes.bfloat16
    HALFB = ROWS // 2
    HALFF = HALFB // 2
    SCE = SC + (SC % 2)
    SCF = SCE // 2
    PH = int(np.lcm(O, P) // P)
    NIPH = (PH * P) // O
    NRT = ROWS // P
    OB = HALFF + SCF
    TOT = HALFF + SCF + NRT * NIPH

    base = np.zeros((P, TOT), dtype=np.float32)
    im_flat = np.ascontiguousarray(im.reshape(ROWS, D))
    base[0:64, :HALFF] = im_flat[:HALFB].T.astype(bf, order='C').view(np.float32)
    base[64:128, :HALFF] = im_flat[HALFB:].T.astype(bf, order='C').view(np.float32)

    # ones region: [128, NRT*NIPH]; col rt*NIPH + j = image j of phase rt//PH
    inv_l = (1.0 / (im_l.astype(np.float64) + EPS)).astype(np.float32)
    g = np.arange(ROWS)
    img = g // O
    rtv = g // P
    rv = g % P
    jv = img - NIPH * (rtv // PH)
    ones = np.zeros((NRT, P, NIPH), dtype=np.float32)
    ones[rtv, rv, jv] = inv_l[img]
    base[:, OB:] = ones.transpose(1, 0, 2).reshape(P, NRT * NIPH)

    in_maps = []
    for m in range(NCORES):
        bm = base.copy()
        sreg = np.zeros((P, SCE), dtype=bf)
        scol = 0
        for (j0, Wt, k, kbs, n) in tiles:
            ids = order[NCORES * np.arange(j0, j0 + n) + m]
            lens = s_l[ids]
            widx = np.minimum(np.arange(Wt)[None, :], (lens - 1)[:, None])
            blk = s[ids[:, None], widx, :]          # [n, Wt, D]
            blkT = blk.reshape(n * Wt, 64).T.astype(bf, order='C')
            sreg[0:64, scol : scol + n * Wt] = blkT    # for row-tile half 0
            sreg[64:128, scol : scol + n * Wt] = blkT  # for row-tile half 1
            scol += n * Wt
        bm[:, HALFF : HALFF + SCF] = sreg.view(np.float32)
        in_maps.append({"inp": bm})
    return order, tiles, SC, in_maps


def _loss_from_scores(full):
    d = np.diag(full).copy()
    t1 = np.maximum(MARGIN + full - d[:, None], 0.0)
    t2 = np.maximum(MARGIN + full - d[None, :], 0.0)
    np.fill_diagonal(t1, 0.0)
    np.fill_diagonal(t2, 0.0)
    return np.float32(t1.mean(dtype=np.float64) + t2.mean(dtype=np.float64))


def kernel(im, s, im_l, s_l):
    from concourse.bass_utils import run_bass_kernel_spmd

    im = np.asarray(im, dtype=np.float32)
    s = np.asarray(s, dtype=np.float32)
    im_l = np.asarray(im_l, dtype=np.int32)
    s_l = np.asarray(s_l, dtype=np.int32)
    I, O, D = im.shape
    C, W, _ = s.shape
    Cc = C // NCORES

    order, tiles, SC, in_maps = _prepare(im, s, im_l, s_l)

    key = (I, O, D, C, W, tuple(int(t[1]) for t in tiles), tuple(int(t[4]) for t in tiles))
    if key not in _CACHE:
        _CACHE[key] = _build_program(I, O, D, Cc, tiles, SC)
    nc = _CACHE[key]

    res = run_bass_kernel_spmd(nc, in_maps, list(range(NCORES)))

    full = np.empty((I, C), dtype=np.float32)
    jr = np.arange(Cc)
    for m in range(NCORES):
        full[:, order[NCORES * jr + m]] = res.results[m]["scores"]
    return _loss_from_scores(full)
